# revision 1
# baseline (speedup 1.0000x reference)
"""Bass/Tile Trainium2 kernel for a 2-layer dense multi-head GAT over a batch
of B=8 independent subgraphs (2048 nodes each, equal contiguous segments).

Sharding: one subgraph per NeuronCore (8 cores), parameters replicated.

Algorithm (per core / subgraph, per attention layer):
  scores are rank-1:  e_ij = leaky_relu(s1_i + s2_j),  s1 = h@a1, s2 = h@a2.
  exp(leaky_relu(t)) is separable through the sign mask M_ij = [s1_i+s2_j>=0]:
      p_ij = M_ij e^{s1_i} e^{s2_j} + (1-M_ij) e^{a s1_i} e^{a s2_j}
  so softmax(e) @ h needs NO N^2 exp work:
      num_i = g_i (M @ u)_i + (vtot - (M @ v))_i          (e^{a s1} cancels in
      u_j = e^{s2_j} [h_j|1],  v_j = e^{a s2_j} [h_j|1],   the Z ratio; g =
      out_i = num_i[:64] / num_i[64]                       e^{(1-a) s1})
  The N^2 work is one DVE compare pass (0/1 mask tiles, exact in bf16) plus
  mask matmuls. The u|v operand streams as bf16 hi + bf16 residual into the
  same PSUM accumulator, giving ~fp32 accuracy at bf16 speed.
"""

from contextlib import ExitStack

import numpy as np

import concourse.bass as bass
import concourse.tile as tile
from concourse import bacc, mybir
from concourse.masks import make_identity

FP = mybir.dt.float32
BF = mybir.dt.bfloat16
AF = mybir.ActivationFunctionType
OP = mybir.AluOpType

B = 8
N = 2048
D = 64
H = 4
ALPHA = 0.2
P = 128
NCH = N // P  # 16 chunks of 128 nodes
DEXT = D + 1  # h plus ones column


def _attention(nc, pools, scratch, s12, s1b, hext, out_cb):
    """Dense-GAT attention layer: out = softmax(lrelu(s1_i+s2_j)) @ h.

    s12:  [P, NCH, 2] SBUF f32 (s1|s2 in node-chunk column layout)
    s1b:  [P, N] SBUF bf16 (s1 replicated across partitions, free dim = node)
    hext: [P, NCH, DEXT] SBUF f32 (h natural, col D == 1.0)
    out_cb(onorm, q): consumes quarter q of the normalized [P, NCH, D] output.
    """
    const, prep, mask_pool, wide, small, psA, psaux = pools

    # --- exponentials (es2/nes02 split per s12 group so chunk-0's u/v tiles
    # don't wait for all 16 s12 columns) ---
    es2 = prep.tile([P, NCH], FP, tag="es2")
    nes02 = prep.tile([P, NCH], FP, tag="nes02")
    g = prep.tile([P, NCH], FP, tag="g")
    for cg in range(4):
        gs = slice(cg * 4, (cg + 1) * 4)
        nc.scalar.activation(es2[:, gs], s12[:, gs, 1], AF.Exp)
        nc.scalar.activation(nes02[:, gs], s12[:, gs, 1], AF.Exp, scale=ALPHA)
        nc.vector.tensor_scalar(nes02[:, gs], nes02[:, gs], -1.0, None,
                                OP.mult)
    nc.scalar.activation(g, s12[:, :, 0], AF.Exp, scale=1.0 - ALPHA)

    # --- u | -v tiles in bf16-hi + bf16-residual split (fp32-accurate matmul
    # at bf16 stream rate): uv_bf + uv_res == e^{s2} hext | -e^{a s2} hext ---
    uv_bf = prep.tile([P, NCH, 2 * DEXT], BF, tag="uv_bf")
    uv_res = prep.tile([P, NCH, 2 * DEXT], BF, tag="uv_res")
    for c in range(NCH):
        for half, es in ((0, es2), (1, nes02)):
            sl = slice(half * DEXT, (half + 1) * DEXT)
            nc.vector.tensor_scalar(uv_bf[:, c, sl], hext[:, c, :],
                                    es[:, c:c + 1], None, OP.mult)
            nc.vector.scalar_tensor_tensor(uv_res[:, c, sl], hext[:, c, :],
                                           es[:, c:c + 1], uv_bf[:, c, sl],
                                           OP.mult, OP.subtract)

    # --- vtot row [1, 130] = [0...0 | sum_j v_j] (bf16 hi+res), used to seed
    # the PSUM accumulators so A[:, DEXT:] = vtot - M@v directly ---
    ones_col_bf = scratch["ones_col_bf"]
    ones_row_bf = scratch["ones_row_bf"]
    vt_ps = psaux.tile([1, DEXT], FP, tag="aux")
    for c in range(NCH):
        nc.tensor.matmul(vt_ps, ones_col_bf, uv_bf[:, c, DEXT:],
                         start=(c == 0), stop=False)
    for c in range(NCH):
        nc.tensor.matmul(vt_ps, ones_col_bf, uv_res[:, c, DEXT:],
                         start=False, stop=(c == NCH - 1))
    # negate (the stream holds -v), split into bf16 hi+res rows, and stack
    # them [2, 130] via a DMA hop (compute engines cannot write partition 1;
    # the hop is off the critical path since seeding happens last) so each
    # accumulator is seeded by a single K=2 matmul
    vrow_bf = prep.tile([1, 2 * DEXT], BF, tag="vrow_bf")
    nc.vector.memset(vrow_bf[:, 0:DEXT], 0.0)
    nc.vector.tensor_scalar(vrow_bf[:, DEXT:], vt_ps, -1.0, None, OP.mult)
    vres = prep.tile([1, DEXT], BF, tag="vres")
    nc.vector.scalar_tensor_tensor(vres, vt_ps, -1.0, vrow_bf[:, DEXT:],
                                   OP.mult, OP.subtract)
    vrow2 = prep.tile([2, 2 * DEXT], BF, tag="vrow2")
    nc.sync.dma_start(out=vrow2[0:1, :], in_=vrow_bf)
    nc.sync.dma_start(out=vrow2[1:2, 0:DEXT],
                      in_=vrow_bf[:, 0:DEXT])
    nc.sync.dma_start(out=vrow2[1:2, DEXT:], in_=vres)

    # --- masked attention matmuls + per-chunk epilogue, quarter-pipelined.
    # Masks are emitted with 3 steps of lookahead so the DVE stays ahead of
    # the PE across quarter boundaries. ---
    nsum_w = wide.tile([P, NCH, DEXT], FP, tag="nsum")
    onorm = wide.tile([P, NCH, D], FP, tag="onorm")
    LOOKAHEAD = 5
    steps = [(q, jc) for q in range(4) for jc in range(NCH)]
    mask_tiles = {}

    def emit_mask(step_idx):
        if step_idx >= len(steps):
            return
        q, jc = steps[step_idx]
        mt = mask_pool.tile([P, 512], BF, tag="mt", name=f"mt{q}_{jc}")
        nc.vector.tensor_scalar(mt, s1b[:, q * 512:(q + 1) * 512],
                                s12[:, jc, 1:2], 0.0, OP.add, OP.is_ge)
        mask_tiles[(q, jc)] = mt

    for i in range(LOOKAHEAD):
        emit_mask(i)

    for q in range(4):  # quarters of the i (destination-node) axis
        A = [psA.tile([P, 2 * DEXT], FP, tag="A", name=f"A{q}_{il}")
             for il in range(4)]
        for jc in range(NCH):
            mt = mask_tiles.pop((q, jc))
            emit_mask(q * NCH + jc + LOOKAHEAD)
            for il in range(4):
                sl = mt[:, il * P:(il + 1) * P]
                nc.tensor.matmul(A[il], sl, uv_bf[:, jc, :],
                                 start=(jc == 0), stop=False)
                nc.tensor.matmul(A[il], sl, uv_res[:, jc, :],
                                 start=False, stop=False)
        # seed vtot last — PSUM accumulation is order-insensitive, and this
        # keeps the (all-chunk) vtot reduction off the attn-start critical path
        for il in range(4):
            nc.tensor.matmul(A[il], ones_row_bf[0:2, :], vrow2, start=False,
                             stop=True)
        qs = slice(q * 4, (q + 1) * 4)
        for il in range(4):
            ic = q * 4 + il
            # A[:, DEXT:] already holds w = vtot - M@v; evacuate on ScalarE,
            # then nsum = g * (M@u) + w on VectorE.
            w = small.tile([P, DEXT], FP, tag="w")
            nc.scalar.copy(w, A[il][:, DEXT:])
            nc.vector.scalar_tensor_tensor(nsum_w[:, ic, :], A[il][:, 0:DEXT],
                                           g[:, ic:ic + 1], w, OP.mult, OP.add)
        # normalize this quarter and hand off
        rz = small.tile([P, 4], FP, tag="rz")
        nc.vector.reciprocal(rz, nsum_w[:, qs, D])
        for k in range(4):
            ic = q * 4 + k
            nc.vector.tensor_scalar(onorm[:, ic, :], nsum_w[:, ic, 0:D],
                                    rz[:, k:k + 1], None, OP.mult)
        out_cb(onorm, q)


def _elu_q(nc, wide, onorm, q, dst_writer):
    """elu over quarter q of onorm [P, NCH, D]; writes via dst_writer(src)."""
    src = onorm[:, q * 4:(q + 1) * 4, :]
    m = wide.tile([P, 4, D], FP, tag="elu_m", name=f"elu_m{q}")
    nc.vector.tensor_scalar(m, src, 0.0, None, OP.min)
    e = wide.tile([P, 4, D], FP, tag="elu_e", name=f"elu_e{q}")
    nc.scalar.activation(e, m, AF.Exp)
    r = wide.tile([P, 4, D], FP, tag="elu_r", name=f"elu_r{q}")
    nc.vector.tensor_scalar(r, src, 0.0, -1.0, OP.max, OP.add)
    dst_writer(r, e)


def _elu_combine(nc, dst, r, e):
    # final elu add
    nc.vector.tensor_tensor(dst, r, e, OP.add)


def build_kernel():
    nc = bacc.Bacc("TRN2", target_bir_lowering=False, debug=False,
                   num_devices=B)

    x = nc.dram_tensor("x", [N, D], FP, kind="ExternalInput")
    W_heads = nc.dram_tensor("W_heads", [H, D, D], FP, kind="ExternalInput")
    a_heads = nc.dram_tensor("a_heads", [H, 2 * D], FP, kind="ExternalInput")
    W_out = nc.dram_tensor("W_out", [H * D, D], FP, kind="ExternalInput")
    a_out = nc.dram_tensor("a_out", [2 * D], FP, kind="ExternalInput")
    out = nc.dram_tensor("out", [N, D], FP, kind="ExternalOutput")

    with tile.TileContext(nc) as tc, ExitStack() as ctx:
        const = ctx.enter_context(tc.tile_pool(name="const", bufs=1))
        prep = ctx.enter_context(tc.tile_pool(name="prep", bufs=3))
        mask_pool = ctx.enter_context(tc.tile_pool(name="mask", bufs=8))
        wide = ctx.enter_context(tc.tile_pool(name="wide", bufs=3))
        small = ctx.enter_context(tc.tile_pool(name="small", bufs=6))
        psA = ctx.enter_context(tc.tile_pool(name="psA", bufs=4, space="PSUM"))
        psaux = ctx.enter_context(tc.tile_pool(name="psaux", bufs=4, space="PSUM"))
        pools = (const, prep, mask_pool, wide, small, psA, psaux)

        ident = const.tile([P, P], FP)
        make_identity(nc, ident)
        ones128 = const.tile([P, P], FP)
        nc.vector.memset(ones128, 1.0)
        ones_col_bf = const.tile([P, 1], BF)
        nc.vector.memset(ones_col_bf, 1.0)
        ones_row_bf = const.tile([2, P], BF)
        nc.vector.memset(ones_row_bf, 1.0)
        scratch = {"ones128": ones128, "ones_col_bf": ones_col_bf,
                   "ones_row_bf": ones_row_bf}

        # ---- load inputs (x in 4 pieces so transposes start early) ----
        x_sb = const.tile([P, NCH, D], FP)
        x_r = x.rearrange("(c p) d -> p c d", p=P)
        for r4 in range(4):
            nc.sync.dma_start(out=x_sb[:, r4 * 4:(r4 + 1) * 4, :],
                              in_=x_r[:, r4 * 4:(r4 + 1) * 4, :])
        Wh = const.tile([64, H, D], FP)
        nc.sync.dma_start(out=Wh, in_=W_heads.rearrange("h k d -> k h d"))
        WhT = const.tile([64, H, D], FP)
        nc.sync.dma_start(out=WhT, in_=W_heads.rearrange("h k d -> d h k"))
        a_sb = const.tile([64, H, 2], FP)
        nc.sync.dma_start(out=a_sb, in_=a_heads.rearrange("h (t k) -> k h t", t=2))
        Wo = const.tile([P, 2, D], FP)
        nc.sync.dma_start(out=Wo, in_=W_out.rearrange("(c k) d -> k c d", k=P))
        WoT = const.tile([64, 2, P], FP)
        nc.sync.dma_start(out=WoT, in_=W_out.rearrange("(c k) d -> d c k", k=P))
        ao = const.tile([64, 2], FP)
        nc.sync.dma_start(out=ao, in_=a_out.rearrange("(t k) -> k t", t=2))

        # ---- xT via PE transposes; bf16 shadow (on GpSimd, in pieces) ----
        xT = const.tile([64, N], FP)
        for c in range(NCH):
            tp = psaux.tile([64, P], FP, tag="aux")
            nc.tensor.transpose(tp, x_sb[:, c, :], ident)
            # alternate evac engines so ACT is free for the head-0 prep chain
            if c % 2 == 0:
                nc.vector.tensor_copy(xT[:, c * P:(c + 1) * P], tp)
            else:
                nc.scalar.copy(xT[:, c * P:(c + 1) * P], tp)
        xT_bf = const.tile([64, N], BF)
        for r in range(4):
            nc.vector.tensor_copy(xT_bf[:, r * 512:(r + 1) * 512],
                                  xT[:, r * 512:(r + 1) * 512])

        # all heads' wa = W_h @ [a1|a2] upfront (re-association: s = x @ wa);
        # only needs the parameter DMAs, so it fills the startup bubble
        wa_all = const.tile([64, H, 2], FP)
        for h in range(H):
            wap = psaux.tile([64, 2], FP, tag="aux", name=f"wap{h}")
            nc.tensor.matmul(wap, WhT[:, h, :], a_sb[:, h, :], start=True,
                             stop=True)
            nc.scalar.copy(wa_all[:, h, :], wap)

        # ---- layer 1: four heads -> xc01/xc23 (split so the layer-2
        # transposes of head-pair 0/1 need not wait for heads 2/3) ----
        xc01 = const.tile([P, NCH, 2, D], FP)
        xc23 = const.tile([P, NCH, 2, D], FP)

        def l1_prep(h):
            wa = wa_all[:, h, :]
            # s12 columns (batched copies, 4 chunks per PSUM tile)
            s12 = prep.tile([P, NCH, 2], FP, tag="s12", name=f"s12_{h}")
            for cg in range(4):
                sp = psaux.tile([P, 8], FP, tag="aux", name=f"sp{h}_{cg}")
                for k in range(4):
                    c = cg * 4 + k
                    nc.tensor.matmul(sp[:, 2 * k:2 * k + 2],
                                     xT[:, c * P:(c + 1) * P], wa,
                                     start=True, stop=True)
                nc.scalar.copy(s12[:, cg * 4:(cg + 1) * 4, :], sp)

            # s1b (bf16, mask input only): s1 row replicated via ones x wa1
            wa1b = prep.tile([64, P], BF, tag="wa1b", name=f"wa1b_{h}")
            nc.vector.tensor_scalar(wa1b, ones128[0:64, :], wa[:, 0:1], None,
                                    OP.mult)
            s1b = prep.tile([P, N], BF, tag="s1b", name=f"s1b_{h}")
            for r in range(4):
                ps = psaux.tile([P, 512], FP, tag="aux")
                nc.tensor.matmul(ps, wa1b, xT_bf[:, r * 512:(r + 1) * 512],
                                 start=True, stop=True)
                nc.scalar.copy(s1b[:, r * 512:(r + 1) * 512], ps)

            # h natural (+ones col)
            hext = prep.tile([P, NCH, DEXT], FP, tag="hext", name=f"hext_{h}")
            nc.vector.memset(hext[:, :, D], 1.0)
            for c in range(NCH):
                hp = psaux.tile([P, D], FP, tag="aux")
                nc.tensor.matmul(hp, xT[:, c * P:(c + 1) * P], Wh[:, h, :],
                                 start=True, stop=True)
                nc.scalar.copy(hext[:, c, 0:D], hp)
            return s12, s1b, hext

        for h in range(H):
            s12, s1b, hext = l1_prep(h)

            def l1_out(onorm, q, h=h):
                xc = xc01 if h < 2 else xc23

                def write(r, e):
                    _elu_combine(nc, xc[:, q * 4:(q + 1) * 4, h % 2, :], r, e)

                _elu_q(nc, wide, onorm, q, write)

            _attention(nc, pools, scratch, s12, s1b, hext, l1_out)

        # ---- transpose xc -> xcT [P, 2, N] (feature-major) + bf16 shadow ----
        xcT = const.tile([P, 2, N], FP)
        for c in range(NCH):
            for kc, xc in ((0, xc01), (1, xc23)):
                tp = psaux.tile([P, P], FP, tag="aux")
                nc.tensor.transpose(tp, xc[:, c, :, :], ident)
                # alternate evac engines: ACT is busy with the last heads'
                # epilogue work in this region
                if (c + kc) % 2 == 0:
                    nc.vector.tensor_copy(xcT[:, kc, c * P:(c + 1) * P], tp)
                else:
                    nc.scalar.copy(xcT[:, kc, c * P:(c + 1) * P], tp)
        xcT_bf = const.tile([P, 2, N], BF)
        for kc in range(2):
            for r in range(2):
                nc.vector.tensor_copy(
                    xcT_bf[:, kc, r * 1024:(r + 1) * 1024],
                    xcT[:, kc, r * 1024:(r + 1) * 1024])

        # ---- layer 2 projections ----
        wa2 = prep.tile([P, 2, 2], FP, tag="wa2")
        for kc in range(2):
            wap = psaux.tile([P, 2], FP, tag="aux", name=f"wap2_{kc}")
            nc.tensor.matmul(wap, WoT[:, kc, :], ao, start=True, stop=True)
            nc.scalar.copy(wa2[:, kc, :], wap)

        s12_2 = prep.tile([P, NCH, 2], FP, tag="s12")
        for cg in range(4):
            sp = psaux.tile([P, 8], FP, tag="aux", name=f"sp2_{cg}")
            for k in range(4):
                c = cg * 4 + k
                for kc in range(2):
                    nc.tensor.matmul(sp[:, 2 * k:2 * k + 2],
                                     xcT[:, kc, c * P:(c + 1) * P],
                                     wa2[:, kc, :],
                                     start=(kc == 0), stop=(kc == 1))
            nc.scalar.copy(s12_2[:, cg * 4:(cg + 1) * 4, :], sp)

        wa1b2 = prep.tile([P, 2, P], BF, tag="wa1b2")
        for kc in range(2):
            nc.vector.tensor_scalar(wa1b2[:, kc, :], ones128, wa2[:, kc, 0:1],
                                    None, OP.mult)
        s1b_2 = prep.tile([P, N], BF, tag="s1b")
        for r in range(4):
            ps = psaux.tile([P, 512], FP, tag="aux")
            for kc in range(2):
                nc.tensor.matmul(ps, wa1b2[:, kc, :],
                                 xcT_bf[:, kc, r * 512:(r + 1) * 512],
                                 start=(kc == 0), stop=(kc == 1))
            nc.scalar.copy(s1b_2[:, r * 512:(r + 1) * 512], ps)

        h2ext = prep.tile([P, NCH, DEXT], FP, tag="hext")
        nc.vector.memset(h2ext[:, :, D], 1.0)
        for c in range(NCH):
            hp = psaux.tile([P, D], FP, tag="aux")
            for kc in range(2):
                nc.tensor.matmul(hp, xcT[:, kc, c * P:(c + 1) * P],
                                 Wo[:, kc, :], start=(kc == 0), stop=(kc == 1))
            nc.scalar.copy(h2ext[:, c, 0:D], hp)

        # ---- layer 2 attention + elu + log_softmax -> out ----
        out_w = const.tile([P, NCH, D], FP)

        out_r = out.rearrange("(c p) d -> p c d", p=P)
        o2_all = const.tile([P, NCH, D], FP)
        esum_all = const.tile([P, NCH], FP)

        def l2_out(onorm, q):
            # per quarter: elu + raw exp-sum (elu output is <= ~20, so exp is
            # fp32-safe without max subtraction); Ln + final subtract deferred
            # so the Exp/Ln ACT tables swap once, not per quarter
            o2 = o2_all[:, q * 4:(q + 1) * 4, :]

            def write(r, e):
                _elu_combine(nc, o2, r, e)

            _elu_q(nc, wide, onorm, q, write)
            escr = wide.tile([P, 4, D], FP, tag="escr", name=f"escr{q}")
            for k in range(4):
                ic = q * 4 + k
                nc.scalar.activation(escr[:, k, :], o2[:, k, :], AF.Exp,
                                     accum_out=esum_all[:, ic:ic + 1])

        _attention(nc, pools, scratch, s12_2, s1b_2, h2ext, l2_out)

        lse = wide.tile([P, NCH], FP, tag="lse")
        nc.scalar.activation(lse, esum_all, AF.Ln)
        lse_b = bass.AP(tensor=lse.tensor, offset=lse.offset,
                        ap=[lse.ap[0], lse.ap[1], [0, D]])
        nc.vector.tensor_tensor(out_w, o2_all, lse_b, OP.subtract)
        nc.sync.dma_start(out=out_r, in_=out_w)

    nc.compile()
    return nc


_NC_CACHE = {}


def _make_runner(nc):
    """Build a cached sharded executable (run_bass_kernel_spmd re-traces
    jax.jit on every call; this jits once and reuses)."""
    import jax
    from jax.sharding import Mesh, PartitionSpec
    try:
        from jax.experimental.shard_map import shard_map
    except ImportError:
        from jax.shard_map import shard_map
    import concourse.mybir as mb
    from concourse import bass2jax

    bass2jax.install_neuronx_cc_hook()

    part_name = nc.partition_id_tensor.name if nc.partition_id_tensor else None
    in_names, out_names, out_avals = [], [], []
    for alloc in nc.m.functions[0].allocations:
        if not isinstance(alloc, mb.MemoryLocationSet):
            continue
        name = alloc.memorylocations[0].name
        if alloc.kind == "ExternalInput":
            if name != part_name:
                in_names.append(name)
        elif alloc.kind == "ExternalOutput":
            out_names.append(name)
            out_avals.append(jax.core.ShapedArray(
                tuple(alloc.tensor_shape), mb.dt.np(alloc.dtype)))
    n_params = len(in_names)
    all_names = in_names + out_names
    if part_name is not None:
        all_names = all_names + [part_name]

    def _body(*args):
        operands = list(args)
        if part_name is not None:
            operands.append(bass2jax.partition_id_tensor())
        return tuple(bass2jax._bass_exec_p.bind(
            *operands, out_avals=tuple(out_avals), in_names=tuple(all_names),
            out_names=tuple(out_names), lowering_input_output_aliases=(),
            sim_require_finite=True, sim_require_nnan=True, nc=nc))

    devices = jax.devices()[:B]
    mesh = Mesh(np.asarray(devices), ("core",))
    n_outs = len(out_names)
    sharded = jax.jit(
        shard_map(_body, mesh=mesh,
                  in_specs=(PartitionSpec("core"),) * (n_params + n_outs),
                  out_specs=(PartitionSpec("core"),) * n_outs,
                  check_rep=False),
        donate_argnums=tuple(range(n_params, n_params + n_outs)),
        keep_unused=True)

    def run(in_maps):
        concat_in = [
            np.concatenate([np.asarray(in_maps[c][nm])[None] for c in range(B)],
                           axis=0).reshape(B * in_maps[0][nm].shape[0],
                                           *in_maps[0][nm].shape[1:])
            for nm in in_names
        ]
        concat_zeros = [
            np.zeros((B * av.shape[0], *av.shape[1:]), av.dtype)
            for av in out_avals
        ]
        out_arrs = sharded(*concat_in, *concat_zeros)
        return [
            {nm: np.asarray(out_arrs[i]).reshape(B, *out_avals[i].shape)[c]
             for i, nm in enumerate(out_names)}
            for c in range(B)
        ]

    return run


def kernel(**inputs):
    h_states = np.ascontiguousarray(np.asarray(inputs["h_states"], dtype=np.float32))
    W_heads = np.ascontiguousarray(np.asarray(inputs["W_heads"], dtype=np.float32))
    a_heads = np.ascontiguousarray(np.asarray(inputs["a_heads"], dtype=np.float32))
    W_out = np.ascontiguousarray(np.asarray(inputs["W_out"], dtype=np.float32))
    a_out = np.ascontiguousarray(np.asarray(inputs["a_out"], dtype=np.float32))

    if "nc" not in _NC_CACHE:
        _NC_CACHE["nc"] = build_kernel()
        _NC_CACHE["run"] = _make_runner(_NC_CACHE["nc"])

    xs = h_states.reshape(B, N, D)
    in_maps = [
        {"x": xs[c], "W_heads": W_heads, "a_heads": a_heads,
         "W_out": W_out, "a_out": a_out}
        for c in range(B)
    ]
    results = _NC_CACHE["run"](in_maps)
    return np.concatenate([results[c]["out"] for c in range(B)], axis=0)


if __name__ == "__main__":
    # smoke test (self-contained: random inputs, shape/dtype check only)
    rng = np.random.default_rng(0)
    inputs = {
        "h_states": rng.standard_normal((B * N, D)).astype(np.float32),
        "W_heads": rng.standard_normal((H, D, D)).astype(np.float32) * 0.18,
        "a_heads": rng.standard_normal((H, 2 * D)).astype(np.float32) * 0.18,
        "W_out": rng.standard_normal((H * D, D)).astype(np.float32) * 0.09,
        "a_out": rng.standard_normal((2 * D,)).astype(np.float32) * 0.18,
        "seq_start_end": (np.arange(B, dtype=np.int32)[:, None] * N
                          + np.array([0, N], dtype=np.int32)[None, :]),
    }
    got = kernel(**inputs)
    print("kernel output", got.shape, got.dtype)



# revision 11
# speedup vs baseline: 1.3001x; 1.3001x over previous
"""Bass/Tile Trainium2 kernel for a 2-layer dense multi-head GAT over a batch
of B=8 independent subgraphs (2048 nodes each, equal contiguous segments).

Sharding: one subgraph per NeuronCore (8 cores), parameters replicated.

Algorithm (per core / subgraph, per attention layer):
  scores are rank-1:  e_ij = leaky_relu(s1_i + s2_j),  s1 = h@a1, s2 = h@a2.
  exp(leaky_relu(t)) is separable through the sign mask M_ij = [s1_i+s2_j>=0]:
      p_ij = M_ij e^{s1_i} e^{s2_j} + (1-M_ij) e^{a s1_i} e^{a s2_j}
  so softmax(e) @ h needs NO N^2 exp work:
      num_i = g_i (M @ u)_i + (vtot - (M @ v))_i          (e^{a s1} cancels in
      u_j = e^{s2_j} [h_j|1],  v_j = e^{a s2_j} [h_j|1],   the Z ratio; g =
      out_i = num_i[:64] / num_i[64]                       e^{(1-a) s1})
  The N^2 work is the 0/1 mask build (exact in bf16) plus mask matmuls with a
  single bf16 [u|-v] stream.  Mask tiles are produced by THREE engines in
  parallel -- DVE (tensor_scalar is_ge, 2x bf16 mode), ACT (sigmoid with a
  2^63 scale saturates to exact 0/1; at the boundary any mask value is exact
  since both leaky-relu branches agree there), with GPSIMD reserved for the
  PSUM epilogue (scalar_tensor_tensor reading both PSUM halves directly).
"""

from contextlib import ExitStack

import numpy as np

import concourse.bass as bass
import concourse.tile as tile
from concourse import bacc, mybir
from concourse.masks import make_identity

FP = mybir.dt.float32
BF = mybir.dt.bfloat16
AF = mybir.ActivationFunctionType
OP = mybir.AluOpType

B = 8
N = 2048
D = 64
H = 4
ALPHA = 0.2
P = 128
NCH = N // P  # 16 chunks of 128 nodes
DEXT = D + 1  # h plus ones column
BIGS = 2.0 ** 63  # exact power of two: sigmoid(BIGS*x) == step(x) in fp32


def _attention(nc, pools, scratch, s12, s1b, hext, out_cb, act_units,
               gp_units, tag):
    """Dense-GAT attention layer: out = softmax(lrelu(s1_i+s2_j)) @ h.

    s12:  [P, NCH, 2] SBUF f32 (s1|s2 in node-chunk column layout)
    s1b:  [P, N] SBUF bf16 (s1 replicated across partitions, free dim = node)
    hext: [P, NCH, DEXT] SBUF bf16 (h natural, col D == 1.0)
    act_units / gp_units: sets of (half, jc) mask units emitted on ACT
        (sigmoid) / GPSIMD (is_ge) as [P,1024] tiles; the rest are DVE
        is_ge tiles ([P,512] per quarter).
    out_cb(onorm, q): consumes quarter q of the normalized [P, NCH, D] output.
    """
    const, prep, mask_dve, mask_act, wide, small, psA, psaux = pools

    # --- exponentials (grouped so chunk-0's uv tiles don't wait on all 16
    # s12 columns) ---
    es2 = prep.tile([P, NCH], FP, tag="es2")
    nes2 = prep.tile([P, NCH], FP, tag="nes2")
    g = prep.tile([P, NCH], FP, tag="g")
    s2big = prep.tile([P, NCH], FP, tag="s2big")
    for cg in range(4):
        gs = slice(cg * 4, (cg + 1) * 4)
        nc.scalar.activation(es2[:, gs], s12[:, gs, 1], AF.Exp)
        nc.scalar.activation(nes2[:, gs], s12[:, gs, 1], AF.Exp, scale=ALPHA)
    nc.scalar.activation(g, s12[:, :, 0], AF.Exp, scale=1.0 - ALPHA)
    nc.vector.tensor_scalar(s2big, s12[:, :, 1], BIGS, None, OP.mult)

    # --- u | -v tiles, single bf16 stream (the -1 fold rides the second
    # tensor_scalar ALU op for free) ---
    uv_bf = prep.tile([P, NCH, 2 * DEXT], BF, tag="uv_bf")
    for c in range(NCH):
        nc.vector.tensor_scalar(uv_bf[:, c, 0:DEXT], hext[:, c, :],
                                es2[:, c:c + 1], None, OP.mult)
        nc.vector.tensor_scalar(uv_bf[:, c, DEXT:], hext[:, c, :],
                                nes2[:, c:c + 1], -1.0, OP.mult, OP.mult)

    # --- vtot row: [0...0 | sum_j v_j] as bf16 hi+res, seeds the PSUM
    # accumulators so A[:, DEXT:] = vtot - M@v directly ---
    ones_col_bf = scratch["ones_col_bf"]
    ones_row_bf = scratch["ones_row_bf"]
    vt_ps = psaux.tile([1, DEXT], FP, tag="aux")
    for c in range(NCH):
        nc.tensor.matmul(vt_ps, ones_col_bf, uv_bf[:, c, DEXT:],
                         start=(c == 0), stop=(c == NCH - 1))
    # negate (the stream holds -v), split into bf16 hi+res rows, and stack
    # them [2, 130] via a DMA hop (compute engines cannot write partition 1;
    # the hop is off the critical path since seeding happens last)
    vrow_bf = prep.tile([1, 2 * DEXT], BF, tag="vrow_bf")
    nc.vector.memset(vrow_bf[:, 0:DEXT], 0.0)
    nc.vector.tensor_scalar(vrow_bf[:, DEXT:], vt_ps, -1.0, None, OP.mult)
    vres = prep.tile([1, DEXT], BF, tag="vres")
    nc.vector.scalar_tensor_tensor(vres, vt_ps, -1.0, vrow_bf[:, DEXT:],
                                   OP.mult, OP.subtract)
    vrow2 = prep.tile([2, 2 * DEXT], BF, tag="vrow2")
    nc.sync.dma_start(out=vrow2[0:1, :], in_=vrow_bf)
    nc.sync.dma_start(out=vrow2[1:2, 0:DEXT], in_=vrow_bf[:, 0:DEXT])
    nc.sync.dma_start(out=vrow2[1:2, DEXT:], in_=vres)

    # --- masked attention matmuls + per-chunk epilogue, quarter-pipelined.
    # Mask units are emitted with lookahead so producers stay ahead of the
    # PE across quarter boundaries. ---
    nsum_w = wide.tile([P, NCH, DEXT], FP, tag="nsum")
    onorm = wide.tile([P, NCH, D], FP, tag="onorm")
    LOOKAHEAD = 6
    steps = [(q, jc) for q in range(4) for jc in range(NCH)]
    tiles = {}  # (q, jc) -> (tile, col_off) of a [*, 512] mask slice

    def emit_step(step_idx):
        if step_idx >= len(steps):
            return
        q, jc = steps[step_idx]
        if (q, jc) in tiles:
            return
        half = q // 2
        if (half, jc) in act_units:
            mt = mask_act.tile([P, 1024], BF, tag="mta",
                               name=f"mta{tag}_{half}_{jc}")
            nc.scalar.activation(mt, s1b[:, half * 1024:(half + 1) * 1024],
                                 AF.Sigmoid, bias=s2big[:, jc:jc + 1],
                                 scale=BIGS)
            tiles[(2 * half, jc)] = (mt, 0)
            tiles[(2 * half + 1, jc)] = (mt, 512)
        elif (half, jc) in gp_units:
            mt = mask_act.tile([P, 1024], BF, tag="mtg",
                               name=f"mtg{tag}_{half}_{jc}")
            nc.gpsimd.tensor_scalar(mt, s1b[:, half * 1024:(half + 1) * 1024],
                                    s12[:, jc, 1:2], 0.0, OP.add, OP.is_ge)
            tiles[(2 * half, jc)] = (mt, 0)
            tiles[(2 * half + 1, jc)] = (mt, 512)
        else:
            mt = mask_dve.tile([P, 512], BF, tag="mtd",
                               name=f"mtd{tag}_{q}_{jc}")
            nc.vector.tensor_scalar(mt, s1b[:, q * 512:(q + 1) * 512],
                                    s12[:, jc, 1:2], 0.0, OP.add, OP.is_ge)
            tiles[(q, jc)] = (mt, 0)

    for i in range(LOOKAHEAD):
        emit_step(i)

    for q in range(4):  # quarters of the i (destination-node) axis
        A = [psA.tile([P, 2 * DEXT], FP, tag="A", name=f"A{tag}_{q}_{il}")
             for il in range(4)]
        for jc in range(NCH):
            mt, off = tiles.pop((q, jc))
            emit_step(q * NCH + jc + LOOKAHEAD)
            for il in range(4):
                sl = mt[:, off + il * P:off + (il + 1) * P]
                nc.tensor.matmul(A[il], sl, uv_bf[:, jc, :],
                                 start=(jc == 0), stop=False)
        # seed vtot last -- PSUM accumulation is order-insensitive, and this
        # keeps the (all-chunk) vtot reduction off the attn-start critical path
        for il in range(4):
            nc.tensor.matmul(A[il], ones_row_bf[0:2, :], vrow2, start=False,
                             stop=True)
        qs = slice(q * 4, (q + 1) * 4)
        for il in range(4):
            ic = q * 4 + il
            # nsum = g * (M@u) + (vtot - M@v): ACT evacuates the u-half with
            # the g-scale fused (Copy+scale), DVE adds the PSUM w-half.
            nU = small.tile([P, DEXT], FP, tag="nU", name=f"nU{tag}_{ic}")
            nc.scalar.activation(nU, A[il][:, 0:DEXT], AF.Copy,
                                 scale=g[:, ic:ic + 1])
            nc.vector.tensor_tensor(nsum_w[:, ic, :], nU, A[il][:, DEXT:],
                                    OP.add)
        # normalize this quarter and hand off
        rz = small.tile([P, 4], FP, tag="rz")
        nc.vector.reciprocal(rz, nsum_w[:, qs, D])
        for k in range(4):
            ic = q * 4 + k
            nc.gpsimd.tensor_scalar(onorm[:, ic, :], nsum_w[:, ic, 0:D],
                                    rz[:, k:k + 1], None, OP.mult)
        out_cb(onorm, q)


def _elu_q(nc, wide, onorm, q, dst_writer):
    """elu over quarter q of onorm [P, NCH, D]; writes via dst_writer(src)."""
    src = onorm[:, q * 4:(q + 1) * 4, :]
    m = wide.tile([P, 4, D], FP, tag="elu_m", name=f"elu_m{q}")
    nc.gpsimd.tensor_scalar(m, src, 0.0, None, OP.min)
    e = wide.tile([P, 4, D], FP, tag="elu_e", name=f"elu_e{q}")
    nc.scalar.activation(e, m, AF.Exp)
    r = wide.tile([P, 4, D], FP, tag="elu_r", name=f"elu_r{q}")
    nc.gpsimd.tensor_scalar(r, src, 0.0, -1.0, OP.max, OP.add)
    dst_writer(r, e)


def _elu_combine(nc, dst, r, e):
    # final elu add
    nc.gpsimd.tensor_tensor(dst, r, e, OP.add)


def build_kernel():
    nc = bacc.Bacc("TRN2", target_bir_lowering=False, debug=False,
                   num_devices=B)

    x = nc.dram_tensor("x", [N, D], FP, kind="ExternalInput")
    W_heads = nc.dram_tensor("W_heads", [H, D, D], FP, kind="ExternalInput")
    a_heads = nc.dram_tensor("a_heads", [H, 2 * D], FP, kind="ExternalInput")
    W_out = nc.dram_tensor("W_out", [H * D, D], FP, kind="ExternalInput")
    a_out = nc.dram_tensor("a_out", [2 * D], FP, kind="ExternalInput")
    out = nc.dram_tensor("out", [N, D], FP, kind="ExternalOutput")

    with tile.TileContext(nc) as tc, ExitStack() as ctx:
        const = ctx.enter_context(tc.tile_pool(name="const", bufs=1))
        prep = ctx.enter_context(tc.tile_pool(name="prep", bufs=3))
        mask_dve = ctx.enter_context(tc.tile_pool(name="mask_dve", bufs=10))
        mask_act = ctx.enter_context(tc.tile_pool(name="mask_act", bufs=5))
        wide = ctx.enter_context(tc.tile_pool(name="wide", bufs=3))
        small = ctx.enter_context(tc.tile_pool(name="small", bufs=6))
        psA = ctx.enter_context(tc.tile_pool(name="psA", bufs=6, space="PSUM"))
        psaux = ctx.enter_context(tc.tile_pool(name="psaux", bufs=2,
                                               space="PSUM"))
        pools = (const, prep, mask_dve, mask_act, wide, small, psA, psaux)

        ident = const.tile([P, P], FP)
        make_identity(nc, ident)
        ones128 = const.tile([P, P], FP)
        nc.vector.memset(ones128, 1.0)
        ones_col_bf = const.tile([P, 1], BF)
        nc.vector.memset(ones_col_bf, 1.0)
        ones_row_bf = const.tile([2, P], BF)
        nc.vector.memset(ones_row_bf, 1.0)
        scratch = {"ones128": ones128, "ones_col_bf": ones_col_bf,
                   "ones_row_bf": ones_row_bf}

        # ---- load inputs (x in 4 pieces so transposes start early) ----
        x_sb = const.tile([P, NCH, D], FP)
        x_r = x.rearrange("(c p) d -> p c d", p=P)
        for r4 in range(4):
            nc.sync.dma_start(out=x_sb[:, r4 * 4:(r4 + 1) * 4, :],
                              in_=x_r[:, r4 * 4:(r4 + 1) * 4, :])
        Wh = const.tile([64, H, D], FP)
        nc.sync.dma_start(out=Wh, in_=W_heads.rearrange("h k d -> k h d"))
        WhT = const.tile([64, H, D], FP)
        nc.sync.dma_start(out=WhT, in_=W_heads.rearrange("h k d -> d h k"))
        a_sb = const.tile([64, H, 2], FP)
        nc.sync.dma_start(out=a_sb, in_=a_heads.rearrange("h (t k) -> k h t", t=2))
        Wo = const.tile([P, 2, D], FP)
        nc.sync.dma_start(out=Wo, in_=W_out.rearrange("(c k) d -> k c d", k=P))
        WoT = const.tile([64, 2, P], FP)
        nc.sync.dma_start(out=WoT, in_=W_out.rearrange("(c k) d -> d c k", k=P))
        ao = const.tile([64, 2], FP)
        nc.sync.dma_start(out=ao, in_=a_out.rearrange("(t k) -> k t", t=2))

        # bf16 shadows of the moving matmul operands (4x cheaper PE rows)
        Wh_bf = const.tile([64, H, D], BF)
        nc.vector.tensor_copy(Wh_bf, Wh)
        Wo_bf = const.tile([P, 2, D], BF)
        nc.vector.tensor_copy(Wo_bf, Wo)

        # ---- xT via PE transposes; bf16 shadow ----
        xT = const.tile([64, N], FP)
        for c in range(NCH):
            tp = psaux.tile([64, P], FP, tag="aux")
            nc.tensor.transpose(tp, x_sb[:, c, :], ident)
            # alternate evac engines so ACT is free for the head-0 prep chain
            if c % 2 == 0:
                nc.vector.tensor_copy(xT[:, c * P:(c + 1) * P], tp)
            else:
                nc.scalar.copy(xT[:, c * P:(c + 1) * P], tp)
        xT_bf = const.tile([64, N], BF)
        for r in range(4):
            nc.vector.tensor_copy(xT_bf[:, r * 512:(r + 1) * 512],
                                  xT[:, r * 512:(r + 1) * 512])

        # all heads' wa = W_h @ [a1|a2] upfront (re-association: s = x @ wa);
        # only needs the parameter DMAs, so it fills the startup bubble
        wa_all = const.tile([64, H, 2], FP)
        for h in range(H):
            wap = psaux.tile([64, 2], FP, tag="aux", name=f"wap{h}")
            nc.tensor.matmul(wap, WhT[:, h, :], a_sb[:, h, :], start=True,
                             stop=True)
            nc.scalar.copy(wa_all[:, h, :], wap)

        # ---- layer 1: four heads -> xc01/xc23 (split so the layer-2
        # transposes of head-pair 0/1 need not wait for heads 2/3) ----
        xc01 = const.tile([P, NCH, 2, D], FP)
        xc23 = const.tile([P, NCH, 2, D], FP)

        def l1_prep(h):
            wa = wa_all[:, h, :]
            # s12 columns (batched copies, 4 chunks per PSUM tile)
            s12 = prep.tile([P, NCH, 2], FP, tag="s12", name=f"s12_{h}")
            for cg in range(4):
                sp = psaux.tile([P, 8], FP, tag="aux", name=f"sp{h}_{cg}")
                for k in range(4):
                    c = cg * 4 + k
                    nc.tensor.matmul(sp[:, 2 * k:2 * k + 2],
                                     xT[:, c * P:(c + 1) * P], wa,
                                     start=True, stop=True)
                nc.scalar.copy(s12[:, cg * 4:(cg + 1) * 4, :], sp)

            # s1b (bf16, mask input only): s1 row replicated via ones x wa1
            wa1b = prep.tile([64, P], BF, tag="wa1b", name=f"wa1b_{h}")
            nc.vector.tensor_scalar(wa1b, ones128[0:64, :], wa[:, 0:1], None,
                                    OP.mult)
            s1b = prep.tile([P, N], BF, tag="s1b", name=f"s1b_{h}")
            for r in range(4):
                ps = psaux.tile([P, 512], FP, tag="aux")
                nc.tensor.matmul(ps, wa1b, xT_bf[:, r * 512:(r + 1) * 512],
                                 start=True, stop=True)
                nc.scalar.copy(s1b[:, r * 512:(r + 1) * 512], ps)

            # h natural (+ones col), bf16, batched 8 chunks per PSUM bank
            hext = prep.tile([P, NCH, DEXT], BF, tag="hext", name=f"hext_{h}")
            nc.vector.memset(hext[:, :, D], 1.0)
            for half in range(2):
                hp = psaux.tile([P, 8, D], FP, tag="aux", name=f"hp{h}_{half}")
                for k in range(8):
                    c = half * 8 + k
                    nc.tensor.matmul(hp[:, k, :], xT_bf[:, c * P:(c + 1) * P],
                                     Wh_bf[:, h, :], start=(k == 0),
                                     stop=(k == 7))
                nc.scalar.copy(hext[:, half * 8:(half + 1) * 8, 0:D], hp)
            return s12, s1b, hext

        ACT_UNITS_L1 = {(0, 2), (0, 8), (0, 13), (1, 5), (1, 10)}
        GP_UNITS_L1 = {(0, 5), (0, 11), (1, 2), (1, 8), (1, 13)}
        for h in range(H):
            s12, s1b, hext = l1_prep(h)

            def l1_out(onorm, q, h=h):
                xc = xc01 if h < 2 else xc23

                def write(r, e):
                    _elu_combine(nc, xc[:, q * 4:(q + 1) * 4, h % 2, :], r, e)

                _elu_q(nc, wide, onorm, q, write)

            _attention(nc, pools, scratch, s12, s1b, hext, l1_out,
                       ACT_UNITS_L1, GP_UNITS_L1, tag=f"h{h}")

        # ---- transpose xc -> xcT [P, 2, N] (feature-major) + bf16 shadow ----
        xcT = const.tile([P, 2, N], FP)
        for c in range(NCH):
            for kc, xc in ((0, xc01), (1, xc23)):
                tp = psaux.tile([P, P], FP, tag="aux")
                nc.tensor.transpose(tp, xc[:, c, :, :], ident)
                # alternate evac engines: ACT is busy with the last heads'
                # epilogue work in this region
                if (c + kc) % 2 == 0:
                    nc.vector.tensor_copy(xcT[:, kc, c * P:(c + 1) * P], tp)
                else:
                    nc.scalar.copy(xcT[:, kc, c * P:(c + 1) * P], tp)
        xcT_bf = const.tile([P, 2, N], BF)
        for kc in range(2):
            for r in range(2):
                nc.vector.tensor_copy(
                    xcT_bf[:, kc, r * 1024:(r + 1) * 1024],
                    xcT[:, kc, r * 1024:(r + 1) * 1024])

        # ---- layer 2 projections ----
        wa2 = prep.tile([P, 2, 2], FP, tag="wa2")
        for kc in range(2):
            wap = psaux.tile([P, 2], FP, tag="aux", name=f"wap2_{kc}")
            nc.tensor.matmul(wap, WoT[:, kc, :], ao, start=True, stop=True)
            nc.scalar.copy(wa2[:, kc, :], wap)

        s12_2 = prep.tile([P, NCH, 2], FP, tag="s12")
        for cg in range(4):
            sp = psaux.tile([P, 8], FP, tag="aux", name=f"sp2_{cg}")
            for k in range(4):
                c = cg * 4 + k
                for kc in range(2):
                    nc.tensor.matmul(sp[:, 2 * k:2 * k + 2],
                                     xcT[:, kc, c * P:(c + 1) * P],
                                     wa2[:, kc, :],
                                     start=(kc == 0), stop=(kc == 1))
            nc.scalar.copy(s12_2[:, cg * 4:(cg + 1) * 4, :], sp)

        wa1b2 = prep.tile([P, 2, P], BF, tag="wa1b2")
        for kc in range(2):
            nc.vector.tensor_scalar(wa1b2[:, kc, :], ones128, wa2[:, kc, 0:1],
                                    None, OP.mult)
        s1b_2 = prep.tile([P, N], BF, tag="s1b")
        for r in range(4):
            ps = psaux.tile([P, 512], FP, tag="aux")
            for kc in range(2):
                nc.tensor.matmul(ps, wa1b2[:, kc, :],
                                 xcT_bf[:, kc, r * 512:(r + 1) * 512],
                                 start=(kc == 0), stop=(kc == 1))
            nc.scalar.copy(s1b_2[:, r * 512:(r + 1) * 512], ps)

        h2ext = prep.tile([P, NCH, DEXT], BF, tag="hext")
        nc.vector.memset(h2ext[:, :, D], 1.0)
        for half in range(2):
            hp = psaux.tile([P, 8, D], FP, tag="aux", name=f"hp2_{half}")
            for k in range(8):
                c = half * 8 + k
                for kc in range(2):
                    nc.tensor.matmul(hp[:, k, :],
                                     xcT_bf[:, kc, c * P:(c + 1) * P],
                                     Wo_bf[:, kc, :],
                                     start=(k == 0 and kc == 0),
                                     stop=(k == 7 and kc == 1))
            nc.scalar.copy(h2ext[:, half * 8:(half + 1) * 8, 0:D], hp)

        # ---- layer 2 attention + elu + log_softmax -> out ----
        out_w = const.tile([P, NCH, D], FP)

        out_r = out.rearrange("(c p) d -> p c d", p=P)
        o2_all = const.tile([P, NCH, D], FP)
        esum_all = const.tile([P, NCH], FP)

        def l2_out(onorm, q):
            # per quarter: elu + raw exp-sum (elu output is <= ~20, so exp is
            # fp32-safe without max subtraction); Ln + final subtract deferred
            # so the Exp/Ln ACT tables swap once, not per quarter
            o2 = o2_all[:, q * 4:(q + 1) * 4, :]

            def write(r, e):
                _elu_combine(nc, o2, r, e)

            _elu_q(nc, wide, onorm, q, write)
            escr = wide.tile([P, 4, D], FP, tag="escr", name=f"escr{q}")
            for k in range(4):
                ic = q * 4 + k
                nc.scalar.activation(escr[:, k, :], o2[:, k, :], AF.Exp,
                                     accum_out=esum_all[:, ic:ic + 1])

        ACT_UNITS_L2 = {(0, 8), (1, 5)}
        GP_UNITS_L2 = {(0, 4), (0, 11), (1, 2), (1, 8), (1, 13)}
        _attention(nc, pools, scratch, s12_2, s1b_2, h2ext, l2_out,
                   ACT_UNITS_L2, GP_UNITS_L2, tag="l2")

        lse = wide.tile([P, NCH], FP, tag="lse")
        nc.scalar.activation(lse, esum_all, AF.Ln)
        lse_b = bass.AP(tensor=lse.tensor, offset=lse.offset,
                        ap=[lse.ap[0], lse.ap[1], [0, D]])
        nc.vector.tensor_tensor(out_w, o2_all, lse_b, OP.subtract)
        nc.sync.dma_start(out=out_r, in_=out_w)

    nc.compile()
    return nc


_NC_CACHE = {}


def _make_runner(nc):
    """Build a cached sharded executable (run_bass_kernel_spmd re-traces
    jax.jit on every call; this jits once and reuses)."""
    import jax
    from jax.sharding import Mesh, PartitionSpec
    try:
        from jax.experimental.shard_map import shard_map
    except ImportError:
        from jax.shard_map import shard_map
    import concourse.mybir as mb
    from concourse import bass2jax

    bass2jax.install_neuronx_cc_hook()

    part_name = nc.partition_id_tensor.name if nc.partition_id_tensor else None
    in_names, out_names, out_avals = [], [], []
    for alloc in nc.m.functions[0].allocations:
        if not isinstance(alloc, mb.MemoryLocationSet):
            continue
        name = alloc.memorylocations[0].name
        if alloc.kind == "ExternalInput":
            if name != part_name:
                in_names.append(name)
        elif alloc.kind == "ExternalOutput":
            out_names.append(name)
            out_avals.append(jax.core.ShapedArray(
                tuple(alloc.tensor_shape), mb.dt.np(alloc.dtype)))
    n_params = len(in_names)
    all_names = in_names + out_names
    if part_name is not None:
        all_names = all_names + [part_name]

    def _body(*args):
        operands = list(args)
        if part_name is not None:
            operands.append(bass2jax.partition_id_tensor())
        return tuple(bass2jax._bass_exec_p.bind(
            *operands, out_avals=tuple(out_avals), in_names=tuple(all_names),
            out_names=tuple(out_names), lowering_input_output_aliases=(),
            sim_require_finite=True, sim_require_nnan=True, nc=nc))

    devices = jax.devices()[:B]
    mesh = Mesh(np.asarray(devices), ("core",))
    n_outs = len(out_names)
    sharded = jax.jit(
        shard_map(_body, mesh=mesh,
                  in_specs=(PartitionSpec("core"),) * (n_params + n_outs),
                  out_specs=(PartitionSpec("core"),) * n_outs,
                  check_rep=False),
        donate_argnums=tuple(range(n_params, n_params + n_outs)),
        keep_unused=True)

    def run(in_maps):
        concat_in = [
            np.concatenate([np.asarray(in_maps[c][nm])[None] for c in range(B)],
                           axis=0).reshape(B * in_maps[0][nm].shape[0],
                                           *in_maps[0][nm].shape[1:])
            for nm in in_names
        ]
        concat_zeros = [
            np.zeros((B * av.shape[0], *av.shape[1:]), av.dtype)
            for av in out_avals
        ]
        out_arrs = sharded(*concat_in, *concat_zeros)
        return [
            {nm: np.asarray(out_arrs[i]).reshape(B, *out_avals[i].shape)[c]
             for i, nm in enumerate(out_names)}
            for c in range(B)
        ]

    return run


def kernel(**inputs):
    h_states = np.ascontiguousarray(np.asarray(inputs["h_states"], dtype=np.float32))
    W_heads = np.ascontiguousarray(np.asarray(inputs["W_heads"], dtype=np.float32))
    a_heads = np.ascontiguousarray(np.asarray(inputs["a_heads"], dtype=np.float32))
    W_out = np.ascontiguousarray(np.asarray(inputs["W_out"], dtype=np.float32))
    a_out = np.ascontiguousarray(np.asarray(inputs["a_out"], dtype=np.float32))

    if "nc" not in _NC_CACHE:
        _NC_CACHE["nc"] = build_kernel()
        _NC_CACHE["run"] = _make_runner(_NC_CACHE["nc"])

    xs = h_states.reshape(B, N, D)
    in_maps = [
        {"x": xs[c], "W_heads": W_heads, "a_heads": a_heads,
         "W_out": W_out, "a_out": a_out}
        for c in range(B)
    ]
    results = _NC_CACHE["run"](in_maps)
    return np.concatenate([results[c]["out"] for c in range(B)], axis=0)


if __name__ == "__main__":
    # smoke test (self-contained: random inputs, shape/dtype check only)
    rng = np.random.default_rng(0)
    inputs = {
        "h_states": rng.standard_normal((B * N, D)).astype(np.float32),
        "W_heads": rng.standard_normal((H, D, D)).astype(np.float32) * 0.18,
        "a_heads": rng.standard_normal((H, 2 * D)).astype(np.float32) * 0.18,
        "W_out": rng.standard_normal((H * D, D)).astype(np.float32) * 0.09,
        "a_out": rng.standard_normal((2 * D,)).astype(np.float32) * 0.18,
        "seq_start_end": (np.arange(B, dtype=np.int32)[:, None] * N
                          + np.array([0, N], dtype=np.int32)[None, :]),
    }
    got = kernel(**inputs)
    print("kernel output", got.shape, got.dtype)
